# revision 14
# baseline (speedup 1.0000x reference)
"""Guided filter (r=40, eps=1e-3) on 8 Trainium2 NeuronCores.

Sharding: pure data-parallel over the batch dim (8 batches -> 8 cores).
Each core processes 3 channel-images of 512x512.

v4 design (strided stage-2):
  a = cov/(var+eps) and b are pointwise functions of 81x81-box-filtered
  fields, so they are smooth on the r=40 scale. Stage 1 computes the four
  box sums (I, p, Ip, II) over EXACT full-res windows but only at a
  stride-4 sample grid (h,w in {4k+2}), via two banded bf16 indicator
  matmuls (box over the partition dim + transpose each time; 0/1 band is
  exact in bf16). Stage 2 normalizes with an exact f32 normC = 1/(nh*nw)
  and runs the elementwise math in f32 on tiny [128,128] tiles.
  Stage 3 evaluates mean_a/mean_b DENSELY from the strided a,b samples:
  one banded matmul per direction whose 0/1 matrix is the 81-window
  indicator on the sample grid (W1), then one with 1/n4(w) folded (W2,
  bf16); the h-direction sample count 1/n4(h) is applied exactly in f32
  as the ACT drain scale. Final combine: out = mean_a*I + mean_b on DVE.

  PSUM->SBUF traffic is ~15 chunk-drains/image (vs 48 for the full-res
  all-matmul design); TensorE work is ~9k cols/image. The kernel is
  memory-bound (~10 MB/core HBM traffic).
"""

import sys
import numpy as np
import ml_dtypes
from contextlib import ExitStack

sys.path.insert(0, "/opt/trn_rl_repo")

import concourse.bass as bass
import concourse.tile as tile
from concourse import bacc, mybir
from concourse.bass_utils import run_bass_kernel_spmd

F32 = mybir.dt.float32
BF16 = mybir.dt.bfloat16
ALU = mybir.AluOpType

R = 40
EPS = 1e-3
HW_ = 512
NB = 4
CH = 3
P = 128
NCORES = 8
S4 = 4
OFF = 2
NS = HW_ // S4          # 128 samples per axis


def _samp_range(j):
    # samples s with grid(s)=4s+OFF within [128j-40, 128j+127+40]
    s0 = max(0, -(-(P * j - R - OFF) // S4))
    s1 = min(NS, (P * j + P - 1 + R - OFF) // S4 + 1)
    return s0, s1


def make_consts():
    idx = np.arange(HW_)
    n1d = (np.minimum(idx + R, HW_ - 1) - np.maximum(idx - R, 0) + 1).astype(
        np.float64)
    grid = np.arange(NS) * S4 + OFF

    # 0/1 band: bandS[k, j*NS+s] = 1 if |(j*128+k) - grid(s)| <= R
    kk = np.arange(P)
    bandS = np.zeros((P, NB * NS), dtype=ml_dtypes.bfloat16)
    for j in range(NB):
        m = (np.abs((j * P + kk)[:, None] - grid[None, :]) <= R)
        bandS[:, j * NS:(j + 1) * NS] = m.astype(ml_dtypes.bfloat16)

    normC = (1.0 / (n1d[grid][:, None] * n1d[grid][None, :])).astype(np.float32)

    W_ind = (np.abs(grid[:, None] - idx[None, :]) <= R)
    n4 = W_ind.sum(axis=0).astype(np.float64)
    W1 = (W_ind * (1.0 / n4)[None, :]).astype(ml_dtypes.bfloat16)
    W2 = W1
    return {"bandS": np.ascontiguousarray(bandS), "normC": normC,
            "W1": np.ascontiguousarray(W1), "W2": np.ascontiguousarray(W2)}


def _img_view(dram_ap, c):
    return dram_ap[c].rearrange("(hb hp) w -> hp hb w", hp=P)


def build_model():
    nc = bacc.Bacc("TRN2", target_bir_lowering=False, debug=False,
                   num_devices=NCORES)
    I_d = nc.dram_tensor("I", [CH, HW_, HW_], BF16, kind="ExternalInput").ap()
    p_d = nc.dram_tensor("p", [CH, HW_, HW_], BF16, kind="ExternalInput").ap()
    bandS_d = nc.dram_tensor("bandS", [P, NB * NS], BF16,
                             kind="ExternalInput").ap()
    normC_d = nc.dram_tensor("normC", [NS, NS], F32, kind="ExternalInput").ap()
    W1_d = nc.dram_tensor("W1", [NS, HW_], BF16, kind="ExternalInput").ap()
    W2_d = nc.dram_tensor("W2", [NS, HW_], BF16, kind="ExternalInput").ap()
    out_d = nc.dram_tensor("out", [CH, HW_, HW_], F32,
                           kind="ExternalOutput").ap()

    with tile.TileContext(nc) as tc:
        with ExitStack() as ctx:
            build_kernel(ctx, tc, I_d, p_d, out_d, bandS_d, normC_d, W1_d,
                         W2_d)
    nc.compile()
    return nc


def build_kernel(ctx, tc, I_d, p_d, out_d, bandS_d, normC_d, W1_d, W2_d):
    nc = tc.nc

    consts = ctx.enter_context(tc.tile_pool(name="consts", bufs=1))
    bandS = consts.tile_from(bandS_d)
    normC = consts.tile_from(normC_d)
    W1 = consts.tile_from(W1_d)
    W2 = consts.tile_from(W2_d)

    pIf = ctx.enter_context(tc.tile_pool(name="If", bufs=3))
    pPf = ctx.enter_context(tc.tile_pool(name="Pf", bufs=3))
    pBf = ctx.enter_context(tc.tile_pool(name="bfp", bufs=3))
    pYb = ctx.enter_context(tc.tile_pool(name="ybp", bufs=2))
    pS2 = ctx.enter_context(tc.tile_pool(name="s2p", bufs=2))
    pT1 = ctx.enter_context(tc.tile_pool(name="t1p", bufs=2))
    pF1 = ctx.enter_context(tc.tile_pool(name="f1p", bufs=2))
    pOut = ctx.enter_context(tc.tile_pool(name="outp", bufs=2))
    pY = ctx.enter_context(tc.tile_pool(name="psy", bufs=2, space="PSUM"))
    pQ = ctx.enter_context(tc.tile_pool(name="psq", bufs=1, space="PSUM"))
    pO1 = ctx.enter_context(tc.tile_pool(name="pso1", bufs=1, space="PSUM"))
    pRR = ctx.enter_context(tc.tile_pool(name="psrr", bufs=1, space="PSUM"))

    # phase-batched emission across channels: engine queues are FIFO, so
    # emitting each phase for all channels keeps every engine fed.
    chan = {}
    for c in range(CH):
        Ib3 = pIf.tile([P, NB, HW_], BF16, tag="If", name=f"Ib3_{c}")
        pb3 = pPf.tile([P, NB, HW_], BF16, tag="pf", name=f"pb3_{c}")
        if c == 0:
            # halve the first channel's transfers so compute starts sooner
            for h2 in range(2):
                blk = slice(2 * h2, 2 * h2 + 2)
                nc.sync.dma_start(Ib3[:, blk, :], _img_view(I_d, c)[:, blk, :])
                nc.sync.dma_start(pb3[:, blk, :], _img_view(p_d, c)[:, blk, :])
        else:
            nc.sync.dma_start(Ib3[:], _img_view(I_d, c))
            nc.sync.dma_start(pb3[:], _img_view(p_d, c))
        chan[c] = {"Ib3": Ib3, "pb3": pb3}

    for c in range(CH):
        d = chan[c]
        Ib = d["Ib3"][:].rearrange("p hb w -> p (hb w)")
        pb = d["pb3"][:].rearrange("p hb w -> p (hb w)")
        Ipb = pBf.tile([P, NB * HW_], BF16, tag="Ipb", name=f"Ipb_{c}")
        IIb = pBf.tile([P, NB * HW_], BF16, tag="IIb", name=f"IIb_{c}")
        if c == 0:
            HH = 2 * HW_
            for h2 in range(2):
                sl2 = slice(h2 * HH, (h2 + 1) * HH)
                nc.vector.tensor_mul(Ipb[:, sl2], Ib[:, sl2], pb[:, sl2])
                nc.vector.tensor_mul(IIb[:, sl2], Ib[:, sl2], Ib[:, sl2])
        else:
            nc.vector.tensor_mul(Ipb[:], Ib, pb)
            nc.vector.tensor_mul(IIb[:], Ib, Ib)
        d["Ipb"], d["IIb"] = Ipb, IIb
        d["Ib"], d["pb"] = Ib, pb

    for c in range(CH):
        d = chan[c]
        q = pQ.tile([P, 4, NS], F32, tag="q", name=f"q_{c}")
        ybs = []
        for t, Xb in enumerate((d["Ib"], d["pb"], d["Ipb"][:], d["IIb"][:])):
            y = pY.tile([P, NB, NS], F32, tag="y", name=f"y_{c}_{t}")
            for i in range(NB):
                for j in range(NB):
                    s0, s1 = _samp_range(j)
                    nc.tensor.matmul(
                        y[:, i, s0:s1],
                        lhsT=Xb[:, j * HW_ + i * P: j * HW_ + i * P + P],
                        rhs=bandS[:, j * NS + s0: j * NS + s1],
                        start=(j == 0), stop=(j == NB - 1))
            yb = pYb.tile([P, NB * NS], BF16, tag=f"yb{t}", name=f"yb{t}_{c}")
            nc.scalar.copy(yb[:], y[:].rearrange("p i s -> p (i s)"))
            ybs.append(yb)
        for t in range(4):
            for i in range(NB):
                s0, s1 = _samp_range(i)
                nc.tensor.matmul(
                    q[:, t, s0:s1],
                    lhsT=ybs[t][:, i * NS:(i + 1) * NS],
                    rhs=bandS[:, i * NS + s0: i * NS + s1],
                    start=(i == 0), stop=(i == NB - 1))
        d["q"] = q

    for c in range(CH):
        d = chan[c]
        q = d["q"]
        mI = pS2.tile([NS, NS], F32, tag="mI", name=f"mI_{c}")
        mp = pS2.tile([NS, NS], F32, tag="mp", name=f"mp_{c}")
        mIp = pS2.tile([NS, NS], F32, tag="mIp", name=f"mIp_{c}")
        mII = pS2.tile([NS, NS], F32, tag="mII", name=f"mII_{c}")
        nc.vector.tensor_mul(mI[:], q[:, 0, :], normC[:])
        nc.vector.tensor_mul(mp[:], q[:, 1, :], normC[:])
        nc.vector.tensor_mul(mIp[:], q[:, 2, :], normC[:])
        nc.vector.tensor_mul(mII[:], q[:, 3, :], normC[:])
        u = pS2.tile([NS, NS], F32, tag="u", name=f"u_{c}")
        cov = pS2.tile([NS, NS], F32, tag="cov", name=f"cov_{c}")
        vv = pS2.tile([NS, NS], F32, tag="vv", name=f"vv_{c}")
        den = pS2.tile([NS, NS], F32, tag="den", name=f"den_{c}")
        rcp = pS2.tile([NS, NS], F32, tag="rcp", name=f"rcp_{c}")
        a_b = pS2.tile([NS, NS], BF16, tag="a_b", name=f"a_b_{c}")
        t2 = pS2.tile([NS, NS], F32, tag="t2", name=f"t2_{c}")
        b_b = pS2.tile([NS, NS], BF16, tag="b_b", name=f"b_b_{c}")
        nc.vector.tensor_mul(u[:], mI[:], mp[:])
        nc.vector.tensor_sub(cov[:], mIp[:], u[:])
        nc.vector.tensor_mul(vv[:], mI[:], mI[:])
        nc.vector.scalar_tensor_tensor(
            den[:], mII[:], EPS, vv[:], op0=ALU.add, op1=ALU.subtract)
        nc.vector.reciprocal_approx_fast(rcp[:], den[:])
        nc.vector.tensor_mul(a_b[:], cov[:], rcp[:])
        nc.vector.tensor_mul(t2[:], a_b[:], mI[:])
        nc.vector.tensor_sub(b_b[:], mp[:], t2[:])
        d["a_b"], d["b_b"] = a_b, b_b

    for c in range(CH):
        d = chan[c]
        t1a = pT1.tile([NS, HW_], BF16, tag="t1a", name=f"t1a_{c}")
        t1b = pT1.tile([NS, HW_], BF16, tag="t1b", name=f"t1b_{c}")
        o1a = pO1.tile([NS, HW_], F32, tag="o1", name=f"o1a_{c}")
        nc.tensor.matmul(o1a[:], lhsT=d["a_b"][:], rhs=W1[:], start=True,
                         stop=True)
        nc.scalar.copy(t1a[:], o1a[:])
        o1b = pO1.tile([NS, HW_], F32, tag="o1", name=f"o1b_{c}")
        nc.tensor.matmul(o1b[:], lhsT=d["b_b"][:], rhs=W1[:], start=True,
                         stop=True)
        nc.scalar.copy(t1b[:], o1b[:])

        out_t = pOut.tile([P, NB, HW_], F32, tag="out", name=f"out_{c}")
        Ibf = d["Ib3"][:].rearrange("p hb w -> p (hb w)")
        outf = out_t[:].rearrange("p hb w -> p (hb w)")
        for j in (0, 2):
            ra2 = pRR.tile([P, 2, HW_], F32, tag="ra2", name=f"ra2_{c}_{j}")
            rb2 = pRR.tile([P, 2, HW_], F32, tag="rb2", name=f"rb2_{c}_{j}")
            for u_ in range(2):
                nc.tensor.matmul(ra2[:, u_, :],
                                 lhsT=t1a[:, (j + u_) * P:(j + u_ + 1) * P],
                                 rhs=W2[:], start=True, stop=True)
                nc.tensor.matmul(rb2[:, u_, :],
                                 lhsT=t1b[:, (j + u_) * P:(j + u_ + 1) * P],
                                 rhs=W2[:], start=True, stop=True)
            sl = slice(j * HW_, (j + 2) * HW_)
            f1 = pF1.tile([P, 2 * HW_], F32, tag="f1", name=f"f1_{c}_{j}")
            nc.vector.tensor_mul(
                f1[:], ra2[:].rearrange("p u w -> p (u w)"), Ibf[:, sl])
            nc.vector.tensor_add(
                outf[:, sl], rb2[:].rearrange("p u w -> p (u w)"), f1[:])
            nc.sync.dma_start(_img_view(out_d, c)[:, j:j + 2, :],
                              out_t[:, j:j + 2, :])


_NC_CACHE = None
LAST_RESULT = None


def _get_model():
    global _NC_CACHE
    if _NC_CACHE is None:
        _NC_CACHE = build_model()
    return _NC_CACHE


def kernel(I, p, _trace=False):
    global LAST_RESULT
    I = np.asarray(I, dtype=np.float32)
    p = np.asarray(p, dtype=np.float32)
    B = I.shape[0]
    assert I.shape == (B, CH, HW_, HW_), I.shape
    nc = _get_model()
    consts = make_consts()
    Ib = I.astype(ml_dtypes.bfloat16)
    pb = p.astype(ml_dtypes.bfloat16)
    in_maps = []
    for k in range(NCORES):
        m = {"I": np.ascontiguousarray(Ib[k]), "p": np.ascontiguousarray(pb[k])}
        m.update(consts)
        in_maps.append(m)
    res = run_bass_kernel_spmd(nc, in_maps, core_ids=list(range(NCORES)),
                               trace=_trace)
    LAST_RESULT = res
    out = np.stack([res.results[k]["out"] for k in range(NCORES)], axis=0)
    return out.astype(np.float32)


if __name__ == "__main__":
    rng = np.random.default_rng(0)
    I = rng.random((8, CH, HW_, HW_), dtype=np.float32)
    p = rng.random((8, CH, HW_, HW_), dtype=np.float32)
    out = kernel(I, p)
    print("out", out.shape, out.dtype, float(out.mean()))
